# revision 53
# baseline (speedup 1.0000x reference)
"""DMTet geometry extraction on 8 Trainium2 NeuronCores.

Pipeline:
  Launch A (device, verts sharded): pos = verts + tanh(deform)/grid_res
      (planar), occupancy (sdf>0) per vertex.
  Host: tet occupancy codes -> valid (crossing) tets -> crossing edges ->
      canonical-sort + dedup (the global `unique` step) -> interp edge list,
      per-tet crossing-edge id groups, rank-remapped triangle-table rows,
      and the random-access row gathers (no fast large-table gather
      primitive exists in this Bass stack, so index-space work is host-side).
  Launch B (device, edges + tets sharded): linear interpolation along
      crossing edges (reciprocal/weights/lerp) and the rank-select
      producing face vertex ids (3-way for 1-tri tets, 4-way for 2-tri).
"""
import os
import numpy as np

import concourse.bacc as bacc
import concourse.bass as bass
import concourse.mybir as mybir
import concourse.tile as tile
from concourse import bass_utils

NCORES = 8
F32 = mybir.dt.float32

TRIANGLE_TABLE = np.array([
    [-1, -1, -1, -1, -1, -1], [1, 0, 2, -1, -1, -1], [4, 0, 3, -1, -1, -1],
    [1, 4, 2, 1, 3, 4], [3, 1, 5, -1, -1, -1], [2, 3, 0, 2, 5, 3],
    [1, 4, 0, 1, 5, 4], [4, 2, 5, -1, -1, -1], [4, 5, 2, -1, -1, -1],
    [4, 1, 0, 4, 5, 1], [3, 2, 0, 3, 5, 2], [1, 3, 5, -1, -1, -1],
    [4, 1, 2, 4, 3, 1], [3, 0, 4, -1, -1, -1], [2, 0, 1, -1, -1, -1],
    [-1, -1, -1, -1, -1, -1]], dtype=np.int32)
NUM_TRI = np.array([0, 1, 1, 2, 1, 2, 2, 1, 1, 2, 2, 1, 2, 1, 1, 0],
                   dtype=np.int32)
EI = np.array([0, 0, 0, 1, 1, 2], dtype=np.int32)
EJ = np.array([1, 2, 3, 2, 3, 3], dtype=np.int32)
_BITS = ((np.arange(16)[:, None] >> np.arange(4)[None, :]) & 1).astype(np.int32)
CROSS = _BITS[:, EI] != _BITS[:, EJ]          # [16, 6] bool
# rank of slot s among the (sorted) crossing slots of config c; -1 elsewhere
RANK = np.full((16, 6), -1, np.int32)
for _c in range(16):
    for _r, _s in enumerate(np.nonzero(CROSS[_c])[0]):
        RANK[_c, _s] = _r
# triangle table remapped to crossing-edge ranks (per config)
TT_RANK = np.where(TRIANGLE_TABLE >= 0,
                   np.take_along_axis(
                       RANK, np.maximum(TRIANGLE_TABLE, 0), axis=1),
                   -1).astype(np.int32)
# every 2-tri config repeats slot0 at slot3 and slot1 at slot5, so the
# device only computes slots [0,1,2,4]; host expands to 6.
_M2 = NUM_TRI == 2
assert (TRIANGLE_TABLE[_M2, 3] == TRIANGLE_TABLE[_M2, 0]).all()
assert (TRIANGLE_TABLE[_M2, 5] == TRIANGLE_TABLE[_M2, 1]).all()
# rank rows are permutations: 1-tri rows of {0,1,2}; 2-tri [0,1,2,4] of {0..3}
assert all(sorted(TT_RANK[c, :3]) == [0, 1, 2]
           for c in range(16) if NUM_TRI[c] == 1)
assert all(sorted(TT_RANK[c, [0, 1, 2, 4]]) == [0, 1, 2, 3]
           for c in range(16) if NUM_TRI[c] == 2)

EXEC_NS = []  # per-launch max-over-cores HW exec time (filled when tracing)


def _run(nc, in_maps):
    trace = os.environ.get("DMTET_TRACE") == "1"
    kw = {}
    if trace:
        kw = dict(trace=True, trace_cores=list(range(NCORES)))
    res = bass_utils.run_bass_kernel_spmd(
        nc, in_maps, core_ids=list(range(NCORES)), **kw)
    if trace:
        EXEC_NS.append(res.exec_time_ns)
    return res.results


def _build_launch_a(rows_pc, scale):
    """Per core (rows_pc % 128 == 0): planar pos x/y/z + occ from planar
    verts/deform components + sdf."""
    P = 128
    C = rows_pc // P
    nc = bacc.Bacc("TRN2", target_bir_lowering=False, debug=False)
    inp = nc.dram_tensor("inp", [7 * rows_pc], F32, kind="ExternalInput").ap()
    outp = nc.dram_tensor("outp", [4 * rows_pc], F32,
                          kind="ExternalOutput").ap()

    assert C % 3 == 0
    cs = C // 3
    inv = inp.rearrange("(k p c) -> p k c", p=P, c=C)
    outv = outp.rearrange("(k p c) -> p k c", p=P, c=C)
    with tile.TileContext(nc) as tc:
        with tc.tile_pool(name="sbuf", bufs=1) as pool:
            it = pool.tile([P, 7 * C], F32)
            ut = pool.tile([P, 4 * C], F32)
            th = pool.tile([P, 3 * C], F32)
            it3 = it[:].rearrange("p (k c) -> p k c", k=7)
            ut3 = ut[:].rearrange("p (k c) -> p k c", k=4)
            th3 = th[:].rearrange("p (k c) -> p k c", k=3)
            # third-size chunks so load / tanh / add / store overlap
            for q in range(3):
                qs = slice(q * cs, (q + 1) * cs)
                nc.sync.dma_start(out=it3[:, :, qs], in_=inv[:, :, qs])
                nc.scalar.activation(th3[:, :, qs], it3[:, 3:6, qs],
                                     mybir.ActivationFunctionType.Tanh)
                nc.vector.tensor_scalar_mul(
                    th3[:, :, qs], th3[:, :, qs], float(scale))
                nc.vector.tensor_tensor(
                    out=ut3[:, 0:3, qs], in0=it3[:, 0:3, qs],
                    in1=th3[:, :, qs], op=mybir.AluOpType.add)
                nc.vector.tensor_scalar(
                    out=ut3[:, 3, qs], in0=it3[:, 6, qs], scalar1=0.0,
                    scalar2=None, op0=mybir.AluOpType.is_gt)
                nc.scalar.dma_start(out=outv[:, :, qs], in_=ut3[:, :, qs])
    nc.compile()
    return nc


def _build_launch_b(mq, f1q, f2q, ci, cf):
    """Per core: interp over 8 planar streams [mq] -> planar vx/vy/vz [mq];
    rank-select faces: (IM3 [f1q,3], TT1 [f1q,3]) -> FO1 [f1q,3] and
    (IM4 [f2q,4], TT2 [f2q,4]) -> FO2 [f2q,4]."""
    P = 128
    nti = mq // (P * ci)
    nc = bacc.Bacc("TRN2", target_bir_lowering=False, debug=False)
    XP = nc.dram_tensor("XP", [mq * 7], F32, kind="ExternalInput").ap()
    U8 = mybir.dt.uint8
    IT1 = nc.dram_tensor("IT1", [f1q * 3], F32, kind="ExternalInput").ap()
    IT2 = nc.dram_tensor("IT2", [f2q * 4], F32, kind="ExternalInput").ap()
    TU1 = nc.dram_tensor("TU1", [f1q * 3], U8, kind="ExternalInput").ap()
    TU2 = nc.dram_tensor("TU2", [f2q * 4], U8, kind="ExternalInput").ap()
    VOP = nc.dram_tensor("VOP", [mq * 3], F32, kind="ExternalOutput").ap()
    FO1 = nc.dram_tensor("FO1", [f1q * 3], F32, kind="ExternalOutput").ap()
    FO2 = nc.dram_tensor("FO2", [f2q * 4], F32, kind="ExternalOutput").ap()

    # block-plane layouts: dram flat = ((t*K + k)*P + p)*c + c_idx
    # XP planes: pax pay paz dx dy dz w1   (out_k = pa_k + w1*d_k)
    xv = XP.rearrange("(t k p c) -> t p k c", k=7, p=P, c=ci)
    vv = VOP.rearrange("(t k p c) -> t p k c", k=3, p=P, c=ci)

    mul, add = mybir.AluOpType.mult, mybir.AluOpType.add
    iseq = mybir.AluOpType.is_equal

    def face_tile(pool, itv, tuv, fov, t, nout, nj):
        itt = pool.tile([P, cf * nj], F32, tag="itt", name="itt")
        nc.gpsimd.dma_start(out=itt[:], in_=itv[t])
        ttu = pool.tile([P, cf * nout], U8, tag="ttu", name="ttu")
        nc.gpsimd.dma_start(out=ttu[:], in_=tuv[t])
        it3 = itt[:].rearrange("p (k c) -> p k c", k=nj)
        ttb = ttu[:]
        fot = pool.tile([P, cf * nout], F32, tag="fot", name="fot")
        tmp = pool.tile([P, cf * nout], F32, tag="tmp", name="tmp")
        # rank rows are permutations of {0..nj-1}, so eq_{nj-1} is implied:
        # out = im_{nj-1} + sum_{j<nj-1} eq_j * (im_j - im_{nj-1})
        dif = pool.tile([P, cf * (nj - 1)], F32, tag="dif", name="dif")
        iml = it3[:, nj - 1, :]
        for j in range(nj - 1):
            nc.vector.tensor_tensor(
                out=dif[:, j * cf:(j + 1) * cf], in0=it3[:, j, :], in1=iml,
                op=mybir.AluOpType.subtract)
        for j in range(nj - 1):
            dj = dif[:, j * cf:(j + 1) * cf].unsqueeze(1).broadcast_to(
                [P, nout, cf])
            dst = fot if j == 0 else tmp
            nc.vector.scalar_tensor_tensor(
                out=dst[:].rearrange("p (k c) -> p k c", k=nout),
                in0=ttb.rearrange("p (k c) -> p k c", k=nout),
                scalar=float(j), in1=dj, op0=iseq, op1=mul)
            if j > 0:
                nc.vector.tensor_tensor(
                    out=fot[:], in0=fot[:], in1=tmp[:], op=add)
        nc.vector.tensor_tensor(
            out=fot[:].rearrange("p (k c) -> p k c", k=nout),
            in0=fot[:].rearrange("p (k c) -> p k c", k=nout),
            in1=iml.unsqueeze(1).broadcast_to([P, nout, cf]), op=add)
        nc.scalar.dma_start(out=fov[t], in_=fot[:].rearrange(
            "p (k c) -> p k c", k=nout))

    nt1 = f1q // (P * cf)
    nt2 = f2q // (P * cf)
    it1v = IT1.rearrange("(t k p c) -> t p k c", k=3, p=P, c=cf)
    tu1v = TU1.rearrange("(t k p c) -> t p k c", k=3, p=P, c=cf)
    fo1v = FO1.rearrange("(t k p c) -> t p k c", k=3, p=P, c=cf)
    it2v = IT2.rearrange("(t k p c) -> t p k c", k=4, p=P, c=cf)
    tu2v = TU2.rearrange("(t k p c) -> t p k c", k=4, p=P, c=cf)
    fo2v = FO2.rearrange("(t k p c) -> t p k c", k=4, p=P, c=cf)
    # face tiles emitted early (own SWDGE load path) so DVE never starves
    face_args = ([(it1v, tu1v, fo1v, i, 3, 3) for i in range(nt1)]
                 + [(it2v, tu2v, fo2v, i, 4, 4) for i in range(nt2)])

    with tile.TileContext(nc) as tc:
        with tc.tile_pool(name="interp", bufs=4) as pool, \
             tc.tile_pool(name="faces", bufs=3) as fpool:
            for t in range(nti):
                xt = pool.tile([P, ci * 7], F32, tag="xt")
                # split the first/last tiles: shorter ramp & tail
                nsub = 8 if t == 0 else (4 if t == nti - 1 else 1)
                cs = ci // nsub
                ot = pool.tile([P, ci * 3], F32, tag="ot", name="ot")
                for q in range(nsub):
                    qsl = slice(q * cs, (q + 1) * cs)
                    nc.sync.dma_start(
                        out=xt[:].rearrange("p (k c) -> p k c", k=7)[:, :, qsl],
                        in_=xv[t][:, :, qsl])
                    xs = [xt[:, i * ci + q * cs: i * ci + (q + 1) * cs]
                          for i in range(7)]
                    w1 = xs[6]
                    for k in range(3):
                        ok = ot[:, k * ci + q * cs: k * ci + (q + 1) * cs]
                        nc.vector.tensor_tensor(
                            out=ok, in0=xs[3 + k], in1=w1, op=mul)
                        nc.vector.tensor_tensor(
                            out=ok, in0=ok, in1=xs[k], op=add)
                    if nsub > 1:
                        nc.scalar.dma_start(
                            out=vv[t][:, :, qsl],
                            in_=ot[:].rearrange(
                                "p (k c) -> p k c", k=3)[:, :, qsl])
                if nsub == 1:
                    nc.scalar.dma_start(
                        out=vv[t],
                        in_=ot[:].rearrange("p (k c) -> p k c", k=3))
                for i, args in enumerate(face_args):
                    if t == min(i, nti - 1):
                        face_tile(fpool, *args)
    nc.compile()
    return nc




def kernel(verts, sdf, deform, indices, grid_res):
    verts = np.ascontiguousarray(verts, dtype=np.float32)
    sdf = np.ascontiguousarray(sdf, dtype=np.float32)
    deform = np.ascontiguousarray(deform, dtype=np.float32)
    indices = np.ascontiguousarray(indices, dtype=np.int32)
    Nv = verts.shape[0]
    scale = 1.0 / float(grid_res)

    # ---------------- Launch A: pos (planar) + occupancy ----------------
    P = 128
    rows_pc = -(-Nv // (NCORES * P * 3)) * P * 3  # per-core rows, 384-aligned
    tot = rows_pc * NCORES
    IN7 = np.zeros((7, tot), np.float32)
    IN7[0:3, :Nv] = verts.T
    IN7[3:6, :Nv] = deform.T
    IN7[6] = -1.0
    IN7[6, :Nv] = sdf
    ncA = _build_launch_a(rows_pc, scale)
    in_maps = [
        {"inp": np.ascontiguousarray(
            IN7[:, c * rows_pc:(c + 1) * rows_pc]).ravel()}
        for c in range(NCORES)]
    resA = _run(ncA, in_maps)
    outs = [resA[c]["outp"].reshape(4, rows_pc) for c in range(NCORES)]
    pos = [np.concatenate([o[k] for o in outs])[:Nv] for k in range(3)]
    occ = np.concatenate([o[3] for o in outs])[:Nv] > 0.5

    # ---------------- Host: codes, edges, dedup ----------------
    occ_f = occ[indices]                                    # [Nt,4]
    tetcode = (occ_f * np.array([1, 2, 4, 8], np.int32)).sum(-1).astype(np.int32)
    valid = (tetcode > 0) & (tetcode < 15)
    vt = indices[valid]
    codes_v = tetcode[valid]
    Fv = len(vt)

    a_full = vt[:, EI]; b_full = vt[:, EJ]
    lo = np.minimum(a_full, b_full).astype(np.int64)
    hi = np.maximum(a_full, b_full).astype(np.int64)
    keys_full = lo * Nv + hi
    crossing = CROSS[codes_v]
    keys_c = keys_full[crossing]

    if len(keys_c) == 0:
        return (np.zeros((0, 3), np.float32), np.zeros((0, 3), np.int32))

    order = np.argsort(keys_c)
    skeys = keys_c[order]
    flag = np.empty(len(skeys), bool); flag[0] = True
    np.not_equal(skeys[1:], skeys[:-1], out=flag[1:])
    group_sorted = np.cumsum(flag) - 1
    inv = np.empty(len(skeys), np.int64)
    inv[order] = group_sorted
    u = skeys[flag]
    M = len(u)
    ua = (u // Nv).astype(np.int64)
    ub = (u % Nv).astype(np.int64)

    invf = inv.astype(np.float32)
    counts = np.where(NUM_TRI[codes_v] == 2, 4, 3).astype(np.int64)
    starts = np.concatenate([[0], np.cumsum(counts)[:-1]])
    ntri = NUM_TRI[codes_v]
    m1 = ntri == 1
    m2 = ntri == 2
    im3 = invf[starts[m1][:, None] + np.arange(3)]          # [n1, 3]
    im4 = invf[starts[m2][:, None] + np.arange(4)]          # [n2, 4]
    ttr = TT_RANK[codes_v]
    tt1 = ttr[m1][:, :3].astype(np.float32)                 # [n1, 3]
    tt2 = ttr[m2][:, [0, 1, 2, 4]].astype(np.float32)       # [n2, 4]
    n1, n2 = len(im3), len(im4)

    # ---------------- Launch B: interp + face rank-select ----------------
    CI, CF = 512, 512
    qi, qf = P * CI, P * CF
    mq = max(1, -(-M // (NCORES * qi))) * qi
    f1q = max(1, -(-n1 // (NCORES * qf))) * qf
    f2q = max(1, -(-n2 // (NCORES * qf))) * qf
    Mp, F1p, F2p = mq * NCORES, f1q * NCORES, f2q * NCORES

    sa = sdf[ua]; sb = sdf[ub]
    dnm = sa - sb                       # exact f32, matching the reference
    XG = np.zeros((7, Mp), np.float32)
    XG[0, :M] = pos[0][ua]; XG[1, :M] = pos[1][ua]; XG[2, :M] = pos[2][ua]
    XG[3, :M] = pos[0][ub] - XG[0, :M]
    XG[4, :M] = pos[1][ub] - XG[1, :M]
    XG[5, :M] = pos[2][ub] - XG[2, :M]
    XG[6, :M] = sa / dnm
    I1G = np.zeros((3, F1p), np.float32); I1G[:, :n1] = im3.T
    I2G = np.zeros((4, F2p), np.float32); I2G[:, :n2] = im4.T
    # tt ranks as u8; pad rows use 255 so no is_equal(j) ever fires
    T1G = np.full((3, F1p), 255, np.uint8); T1G[:, :n1] = tt1.T
    T2G = np.full((4, F2p), 255, np.uint8); T2G[:, :n2] = tt2.T

    def pack(g, per_core, c, cw):
        nt = per_core // (P * cw)
        sl = g[:, c * per_core:(c + 1) * per_core]
        return np.ascontiguousarray(
            sl.reshape(len(g), nt, P, cw).transpose(1, 0, 2, 3)).ravel()

    ncB = _build_launch_b(mq, f1q, f2q, CI, CF)
    in_maps = [{"XP": pack(XG, mq, c, CI),
                "IT1": pack(I1G, f1q, c, CF),
                "IT2": pack(I2G, f2q, c, CF),
                "TU1": pack(T1G, f1q, c, CF),
                "TU2": pack(T2G, f2q, c, CF)}
               for c in range(NCORES)]
    resB = _run(ncB, in_maps)

    def unpack(name, per_core, k, cw):
        nt = per_core // (P * cw)
        return np.concatenate(
            [resB[c][name].reshape(nt, k, P * cw).transpose(0, 2, 1).reshape(
                -1, k) for c in range(NCORES)])

    verts_out = np.ascontiguousarray(unpack("VOP", mq, 3, CI)[:M])
    f1 = unpack("FO1", f1q, 3, CF)[:n1].astype(np.int32)
    q = unpack("FO2", f2q, 4, CF)[:n2].astype(np.int32)
    f2 = np.empty((n2, 6), np.int32)
    f2[:, 0] = q[:, 0]; f2[:, 1] = q[:, 1]; f2[:, 2] = q[:, 2]
    f2[:, 3] = q[:, 0]; f2[:, 4] = q[:, 3]; f2[:, 5] = q[:, 1]
    faces = np.concatenate([f1, f2.reshape(-1, 3)], axis=0)
    return (verts_out, faces)


# revision 54
# speedup vs baseline: 1.0427x; 1.0427x over previous
"""DMTet geometry extraction on 8 Trainium2 NeuronCores.

Pipeline:
  Launch A (device, verts sharded): pos = verts + tanh(deform)/grid_res
      (planar), occupancy (sdf>0) per vertex.
  Host: tet occupancy codes -> valid (crossing) tets -> crossing edges ->
      canonical-sort + dedup (the global `unique` step) -> interp edge list,
      per-tet crossing-edge id groups, rank-remapped triangle-table rows,
      and the random-access row gathers (no fast large-table gather
      primitive exists in this Bass stack, so index-space work is host-side).
  Launch B (device, edges + tets sharded): linear interpolation along
      crossing edges (reciprocal/weights/lerp) and the rank-select
      producing face vertex ids (3-way for 1-tri tets, 4-way for 2-tri).
"""
import os
import numpy as np

import concourse.bacc as bacc
import concourse.bass as bass
import concourse.mybir as mybir
import concourse.tile as tile
from concourse import bass_utils

NCORES = 8
F32 = mybir.dt.float32

TRIANGLE_TABLE = np.array([
    [-1, -1, -1, -1, -1, -1], [1, 0, 2, -1, -1, -1], [4, 0, 3, -1, -1, -1],
    [1, 4, 2, 1, 3, 4], [3, 1, 5, -1, -1, -1], [2, 3, 0, 2, 5, 3],
    [1, 4, 0, 1, 5, 4], [4, 2, 5, -1, -1, -1], [4, 5, 2, -1, -1, -1],
    [4, 1, 0, 4, 5, 1], [3, 2, 0, 3, 5, 2], [1, 3, 5, -1, -1, -1],
    [4, 1, 2, 4, 3, 1], [3, 0, 4, -1, -1, -1], [2, 0, 1, -1, -1, -1],
    [-1, -1, -1, -1, -1, -1]], dtype=np.int32)
NUM_TRI = np.array([0, 1, 1, 2, 1, 2, 2, 1, 1, 2, 2, 1, 2, 1, 1, 0],
                   dtype=np.int32)
EI = np.array([0, 0, 0, 1, 1, 2], dtype=np.int32)
EJ = np.array([1, 2, 3, 2, 3, 3], dtype=np.int32)
_BITS = ((np.arange(16)[:, None] >> np.arange(4)[None, :]) & 1).astype(np.int32)
CROSS = _BITS[:, EI] != _BITS[:, EJ]          # [16, 6] bool
# rank of slot s among the (sorted) crossing slots of config c; -1 elsewhere
RANK = np.full((16, 6), -1, np.int32)
for _c in range(16):
    for _r, _s in enumerate(np.nonzero(CROSS[_c])[0]):
        RANK[_c, _s] = _r
# triangle table remapped to crossing-edge ranks (per config)
TT_RANK = np.where(TRIANGLE_TABLE >= 0,
                   np.take_along_axis(
                       RANK, np.maximum(TRIANGLE_TABLE, 0), axis=1),
                   -1).astype(np.int32)
# every 2-tri config repeats slot0 at slot3 and slot1 at slot5, so the
# device only computes slots [0,1,2,4]; host expands to 6.
_M2 = NUM_TRI == 2
assert (TRIANGLE_TABLE[_M2, 3] == TRIANGLE_TABLE[_M2, 0]).all()
assert (TRIANGLE_TABLE[_M2, 5] == TRIANGLE_TABLE[_M2, 1]).all()
# rank rows are permutations: 1-tri rows of {0,1,2}; 2-tri [0,1,2,4] of {0..3}
assert all(sorted(TT_RANK[c, :3]) == [0, 1, 2]
           for c in range(16) if NUM_TRI[c] == 1)
assert all(sorted(TT_RANK[c, [0, 1, 2, 4]]) == [0, 1, 2, 3]
           for c in range(16) if NUM_TRI[c] == 2)

EXEC_NS = []  # per-launch max-over-cores HW exec time (filled when tracing)


def _run(nc, in_maps):
    trace = os.environ.get("DMTET_TRACE") == "1"
    kw = {}
    if trace:
        kw = dict(trace=True, trace_cores=list(range(NCORES)))
    res = bass_utils.run_bass_kernel_spmd(
        nc, in_maps, core_ids=list(range(NCORES)), **kw)
    if trace:
        EXEC_NS.append(res.exec_time_ns)
    return res.results


def _build_launch_a(rows_pc, scale):
    """Per core (rows_pc % 128 == 0): planar pos x/y/z + occ from planar
    verts/deform components + sdf."""
    P = 128
    C = rows_pc // P
    nc = bacc.Bacc("TRN2", target_bir_lowering=False, debug=False)
    inp = nc.dram_tensor("inp", [7 * rows_pc], F32, kind="ExternalInput").ap()
    outp = nc.dram_tensor("outp", [4 * rows_pc], F32,
                          kind="ExternalOutput").ap()

    assert C % 3 == 0
    cs = C // 3
    inv = inp.rearrange("(k p c) -> p k c", p=P, c=C)
    outv = outp.rearrange("(k p c) -> p k c", p=P, c=C)
    with tile.TileContext(nc) as tc:
        with tc.tile_pool(name="sbuf", bufs=1) as pool:
            it = pool.tile([P, 7 * C], F32)
            ut = pool.tile([P, 4 * C], F32)
            th = pool.tile([P, 3 * C], F32)
            it3 = it[:].rearrange("p (k c) -> p k c", k=7)
            ut3 = ut[:].rearrange("p (k c) -> p k c", k=4)
            th3 = th[:].rearrange("p (k c) -> p k c", k=3)
            # third-size chunks so load / tanh / add / store overlap
            for q in range(3):
                qs = slice(q * cs, (q + 1) * cs)
                nc.sync.dma_start(out=it3[:, :, qs], in_=inv[:, :, qs])
                nc.scalar.activation(th3[:, :, qs], it3[:, 3:6, qs],
                                     mybir.ActivationFunctionType.Tanh)
                nc.vector.tensor_scalar_mul(
                    th3[:, :, qs], th3[:, :, qs], float(scale))
                nc.vector.tensor_tensor(
                    out=ut3[:, 0:3, qs], in0=it3[:, 0:3, qs],
                    in1=th3[:, :, qs], op=mybir.AluOpType.add)
                nc.vector.tensor_scalar(
                    out=ut3[:, 3, qs], in0=it3[:, 6, qs], scalar1=0.0,
                    scalar2=None, op0=mybir.AluOpType.is_gt)
                nc.scalar.dma_start(out=outv[:, :, qs], in_=ut3[:, :, qs])
    nc.compile()
    return nc


def _build_launch_b(mq, f1q, f2q, ci, cf):
    """Per core: interp over 8 planar streams [mq] -> planar vx/vy/vz [mq];
    rank-select faces: (IM3 [f1q,3], TT1 [f1q,3]) -> FO1 [f1q,3] and
    (IM4 [f2q,4], TT2 [f2q,4]) -> FO2 [f2q,4]."""
    P = 128
    nti = mq // (P * ci)
    nc = bacc.Bacc("TRN2", target_bir_lowering=False, debug=False)
    XP = nc.dram_tensor("XP", [mq * 7], F32, kind="ExternalInput").ap()
    U8 = mybir.dt.uint8
    IT1 = nc.dram_tensor("IT1", [f1q * 3], F32, kind="ExternalInput").ap()
    IT2 = nc.dram_tensor("IT2", [f2q * 4], F32, kind="ExternalInput").ap()
    TU1 = nc.dram_tensor("TU1", [f1q * 3], U8, kind="ExternalInput").ap()
    TU2 = nc.dram_tensor("TU2", [f2q * 4], U8, kind="ExternalInput").ap()
    VOP = nc.dram_tensor("VOP", [mq * 3], F32, kind="ExternalOutput").ap()
    FO1 = nc.dram_tensor("FO1", [f1q * 3], F32, kind="ExternalOutput").ap()
    FO2 = nc.dram_tensor("FO2", [f2q * 4], F32, kind="ExternalOutput").ap()

    # block-plane layouts: dram flat = ((t*K + k)*P + p)*c + c_idx
    # XP planes: pax pay paz dx dy dz w1   (out_k = pa_k + w1*d_k)
    xv = XP.rearrange("(t k p c) -> t p k c", k=7, p=P, c=ci)
    vv = VOP.rearrange("(t k p c) -> t p k c", k=3, p=P, c=ci)

    mul, add = mybir.AluOpType.mult, mybir.AluOpType.add
    iseq = mybir.AluOpType.is_equal

    def face_tile(pool, itv, tuv, fov, t, nout, nj):
        itt = pool.tile([P, cf * nj], F32, tag="itt", name="itt")
        nc.gpsimd.dma_start(out=itt[:], in_=itv[t])
        ttu = pool.tile([P, cf * nout], U8, tag="ttu", name="ttu")
        nc.gpsimd.dma_start(out=ttu[:], in_=tuv[t])
        it3 = itt[:].rearrange("p (k c) -> p k c", k=nj)
        ttb = ttu[:]
        fot = pool.tile([P, cf * nout], F32, tag="fot", name="fot")
        tmp = pool.tile([P, cf * nout], F32, tag="tmp", name="tmp")
        # rank rows are permutations of {0..nj-1}, so eq_{nj-1} is implied:
        # out = im_{nj-1} + sum_{j<nj-1} eq_j * (im_j - im_{nj-1})
        dif = pool.tile([P, cf * (nj - 1)], F32, tag="dif", name="dif")
        iml = it3[:, nj - 1, :]
        for j in range(nj - 1):
            nc.vector.tensor_tensor(
                out=dif[:, j * cf:(j + 1) * cf], in0=it3[:, j, :], in1=iml,
                op=mybir.AluOpType.subtract)
        for j in range(nj - 1):
            dj = dif[:, j * cf:(j + 1) * cf].unsqueeze(1).broadcast_to(
                [P, nout, cf])
            dst = fot if j == 0 else tmp
            nc.vector.scalar_tensor_tensor(
                out=dst[:].rearrange("p (k c) -> p k c", k=nout),
                in0=ttb.rearrange("p (k c) -> p k c", k=nout),
                scalar=float(j), in1=dj, op0=iseq, op1=mul)
            if j > 0:
                nc.vector.tensor_tensor(
                    out=fot[:], in0=fot[:], in1=tmp[:], op=add)
        nc.vector.tensor_tensor(
            out=fot[:].rearrange("p (k c) -> p k c", k=nout),
            in0=fot[:].rearrange("p (k c) -> p k c", k=nout),
            in1=iml.unsqueeze(1).broadcast_to([P, nout, cf]), op=add)
        nc.scalar.dma_start(out=fov[t], in_=fot[:].rearrange(
            "p (k c) -> p k c", k=nout))

    nt1 = f1q // (P * cf)
    nt2 = f2q // (P * cf)
    it1v = IT1.rearrange("(t k p c) -> t p k c", k=3, p=P, c=cf)
    tu1v = TU1.rearrange("(t k p c) -> t p k c", k=3, p=P, c=cf)
    fo1v = FO1.rearrange("(t k p c) -> t p k c", k=3, p=P, c=cf)
    it2v = IT2.rearrange("(t k p c) -> t p k c", k=4, p=P, c=cf)
    tu2v = TU2.rearrange("(t k p c) -> t p k c", k=4, p=P, c=cf)
    fo2v = FO2.rearrange("(t k p c) -> t p k c", k=4, p=P, c=cf)
    # face tiles emitted early (own SWDGE load path) so DVE never starves
    face_args = ([(it1v, tu1v, fo1v, i, 3, 3) for i in range(nt1)]
                 + [(it2v, tu2v, fo2v, i, 4, 4) for i in range(nt2)])

    with tile.TileContext(nc) as tc:
        with tc.tile_pool(name="interp", bufs=4) as pool, \
             tc.tile_pool(name="faces", bufs=3) as fpool:
            for t in range(nti):
                xt = pool.tile([P, ci * 7], F32, tag="xt")
                # quarter the first/last tiles: shorter ramp & tail
                nsub = 4 if t in (0, nti - 1) else 1
                cs = ci // nsub
                ot = pool.tile([P, ci * 3], F32, tag="ot", name="ot")
                for q in range(nsub):
                    qsl = slice(q * cs, (q + 1) * cs)
                    nc.sync.dma_start(
                        out=xt[:].rearrange("p (k c) -> p k c", k=7)[:, :, qsl],
                        in_=xv[t][:, :, qsl])
                    xs = [xt[:, i * ci + q * cs: i * ci + (q + 1) * cs]
                          for i in range(7)]
                    w1 = xs[6]
                    for k in range(3):
                        ok = ot[:, k * ci + q * cs: k * ci + (q + 1) * cs]
                        nc.vector.tensor_tensor(
                            out=ok, in0=xs[3 + k], in1=w1, op=mul)
                        nc.vector.tensor_tensor(
                            out=ok, in0=ok, in1=xs[k], op=add)
                    if nsub > 1:
                        nc.scalar.dma_start(
                            out=vv[t][:, :, qsl],
                            in_=ot[:].rearrange(
                                "p (k c) -> p k c", k=3)[:, :, qsl])
                if nsub == 1:
                    nc.scalar.dma_start(
                        out=vv[t],
                        in_=ot[:].rearrange("p (k c) -> p k c", k=3))
                for i, args in enumerate(face_args):
                    if t == min(i, nti - 1):
                        face_tile(fpool, *args)
    nc.compile()
    return nc




def kernel(verts, sdf, deform, indices, grid_res):
    verts = np.ascontiguousarray(verts, dtype=np.float32)
    sdf = np.ascontiguousarray(sdf, dtype=np.float32)
    deform = np.ascontiguousarray(deform, dtype=np.float32)
    indices = np.ascontiguousarray(indices, dtype=np.int32)
    Nv = verts.shape[0]
    scale = 1.0 / float(grid_res)

    # ---------------- Launch A: pos (planar) + occupancy ----------------
    P = 128
    rows_pc = -(-Nv // (NCORES * P * 3)) * P * 3  # per-core rows, 384-aligned
    tot = rows_pc * NCORES
    IN7 = np.zeros((7, tot), np.float32)
    IN7[0:3, :Nv] = verts.T
    IN7[3:6, :Nv] = deform.T
    IN7[6] = -1.0
    IN7[6, :Nv] = sdf
    ncA = _build_launch_a(rows_pc, scale)
    in_maps = [
        {"inp": np.ascontiguousarray(
            IN7[:, c * rows_pc:(c + 1) * rows_pc]).ravel()}
        for c in range(NCORES)]
    resA = _run(ncA, in_maps)
    outs = [resA[c]["outp"].reshape(4, rows_pc) for c in range(NCORES)]
    pos = [np.concatenate([o[k] for o in outs])[:Nv] for k in range(3)]
    occ = np.concatenate([o[3] for o in outs])[:Nv] > 0.5

    # ---------------- Host: codes, edges, dedup ----------------
    occ_f = occ[indices]                                    # [Nt,4]
    tetcode = (occ_f * np.array([1, 2, 4, 8], np.int32)).sum(-1).astype(np.int32)
    valid = (tetcode > 0) & (tetcode < 15)
    vt = indices[valid]
    codes_v = tetcode[valid]
    Fv = len(vt)

    a_full = vt[:, EI]; b_full = vt[:, EJ]
    lo = np.minimum(a_full, b_full).astype(np.int64)
    hi = np.maximum(a_full, b_full).astype(np.int64)
    keys_full = lo * Nv + hi
    crossing = CROSS[codes_v]
    keys_c = keys_full[crossing]

    if len(keys_c) == 0:
        return (np.zeros((0, 3), np.float32), np.zeros((0, 3), np.int32))

    order = np.argsort(keys_c)
    skeys = keys_c[order]
    flag = np.empty(len(skeys), bool); flag[0] = True
    np.not_equal(skeys[1:], skeys[:-1], out=flag[1:])
    group_sorted = np.cumsum(flag) - 1
    inv = np.empty(len(skeys), np.int64)
    inv[order] = group_sorted
    u = skeys[flag]
    M = len(u)
    ua = (u // Nv).astype(np.int64)
    ub = (u % Nv).astype(np.int64)

    invf = inv.astype(np.float32)
    counts = np.where(NUM_TRI[codes_v] == 2, 4, 3).astype(np.int64)
    starts = np.concatenate([[0], np.cumsum(counts)[:-1]])
    ntri = NUM_TRI[codes_v]
    m1 = ntri == 1
    m2 = ntri == 2
    im3 = invf[starts[m1][:, None] + np.arange(3)]          # [n1, 3]
    im4 = invf[starts[m2][:, None] + np.arange(4)]          # [n2, 4]
    ttr = TT_RANK[codes_v]
    tt1 = ttr[m1][:, :3].astype(np.float32)                 # [n1, 3]
    tt2 = ttr[m2][:, [0, 1, 2, 4]].astype(np.float32)       # [n2, 4]
    n1, n2 = len(im3), len(im4)

    # ---------------- Launch B: interp + face rank-select ----------------
    CI, CF = 512, 512
    qi, qf = P * CI, P * CF
    mq = max(1, -(-M // (NCORES * qi))) * qi
    f1q = max(1, -(-n1 // (NCORES * qf))) * qf
    f2q = max(1, -(-n2 // (NCORES * qf))) * qf
    Mp, F1p, F2p = mq * NCORES, f1q * NCORES, f2q * NCORES

    sa = sdf[ua]; sb = sdf[ub]
    dnm = sa - sb                       # exact f32, matching the reference
    XG = np.zeros((7, Mp), np.float32)
    XG[0, :M] = pos[0][ua]; XG[1, :M] = pos[1][ua]; XG[2, :M] = pos[2][ua]
    XG[3, :M] = pos[0][ub] - XG[0, :M]
    XG[4, :M] = pos[1][ub] - XG[1, :M]
    XG[5, :M] = pos[2][ub] - XG[2, :M]
    XG[6, :M] = sa / dnm
    I1G = np.zeros((3, F1p), np.float32); I1G[:, :n1] = im3.T
    I2G = np.zeros((4, F2p), np.float32); I2G[:, :n2] = im4.T
    # tt ranks as u8; pad rows use 255 so no is_equal(j) ever fires
    T1G = np.full((3, F1p), 255, np.uint8); T1G[:, :n1] = tt1.T
    T2G = np.full((4, F2p), 255, np.uint8); T2G[:, :n2] = tt2.T

    def pack(g, per_core, c, cw):
        nt = per_core // (P * cw)
        sl = g[:, c * per_core:(c + 1) * per_core]
        return np.ascontiguousarray(
            sl.reshape(len(g), nt, P, cw).transpose(1, 0, 2, 3)).ravel()

    ncB = _build_launch_b(mq, f1q, f2q, CI, CF)
    in_maps = [{"XP": pack(XG, mq, c, CI),
                "IT1": pack(I1G, f1q, c, CF),
                "IT2": pack(I2G, f2q, c, CF),
                "TU1": pack(T1G, f1q, c, CF),
                "TU2": pack(T2G, f2q, c, CF)}
               for c in range(NCORES)]
    resB = _run(ncB, in_maps)

    def unpack(name, per_core, k, cw):
        nt = per_core // (P * cw)
        return np.concatenate(
            [resB[c][name].reshape(nt, k, P * cw).transpose(0, 2, 1).reshape(
                -1, k) for c in range(NCORES)])

    verts_out = np.ascontiguousarray(unpack("VOP", mq, 3, CI)[:M])
    f1 = unpack("FO1", f1q, 3, CF)[:n1].astype(np.int32)
    q = unpack("FO2", f2q, 4, CF)[:n2].astype(np.int32)
    f2 = np.empty((n2, 6), np.int32)
    f2[:, 0] = q[:, 0]; f2[:, 1] = q[:, 1]; f2[:, 2] = q[:, 2]
    f2[:, 3] = q[:, 0]; f2[:, 4] = q[:, 3]; f2[:, 5] = q[:, 1]
    faces = np.concatenate([f1, f2.reshape(-1, 3)], axis=0)
    return (verts_out, faces)
